# revision 8
# baseline (speedup 1.0000x reference)
"""Trainium2 Bass kernel for nn_CustomLoss_69999376990919.

Math: the reference's A-inner-product modified Gram-Schmidt + projection
collapses to per-sample 4x4 Gram matrices
    G[s] = P_s diag(a_s) P_s^T,   R[s] = P_s diag(a_s) T_s
after which   loss = mean_s (4 - tr(R^T G^{-1} R)) / 4
(Cholesky of G == Gram-Schmidt in exact arithmetic; <v,Av> > 0 always holds
since coefficients > 0).  The device streams all inputs (memory-bound) and
produces G/R; the tiny 4x4 solves run on the host in float64.

Sharding: pure data parallelism, batch axis 0 split across 8 cores
(64 samples each).  Per core, samples run in 4 groups of 16.
Layout: n = p*128 + f (p = SBUF partition, f = free chunk).  Per f-chunk,
ONE bf16 matmul accumulates G and R for all 16 samples jointly:
  lhsT = W(f) = (a*P)(f) as [128, (i,s)] stationary (64 cols),
  rhs  = U(f) = [P | T^t](f) as [128, (s,x)] moving (128 cols, x = 4 pred
         classes then 4 target columns),
  PSUM[(i,s), (s',x)] accumulated over the 128 f-chunks; the s==s' block
diagonals are the per-sample G/R entries (extracted on host).
T is DMA'd in its natural (f,m)-contiguous layout and transposed into U's
(m,f) layout on-chip (DVE/GpSimd alternating).  bf16 is safe: the loss is
1 - O(1e-4); bf16-quantized inputs move the final scalar by ~1e-9 relative.
"""

import os
from contextlib import ExitStack

import numpy as np

import concourse.bacc as bacc
import concourse.bass as bass
import concourse.tile as tile
from concourse import mybir
from concourse.bass_utils import run_bass_kernel_spmd

B, C, N = 512, 4, 16384
H = 0.0078125  # grid spacing; A = diag(h^2 * coefficients)
NCORES = 8
SPC = B // NCORES  # 64 samples per core
GS = 16            # samples per group
NG = SPC // GS     # 4 groups per core
P = 128            # SBUF partitions; n = p*128 + f
F = N // P         # 128 f-chunks
X = 2 * C          # rhs columns per (s, f): 4 preds + 4 targets
QP = C * GS        # psum partitions (i, s)

_CACHE = {}


def _build_bass():
    nc = bacc.Bacc(trn_type="TRN2")
    coeff = nc.dram_tensor("coeff", [SPC, N], mybir.dt.float32, kind="ExternalInput")
    preds = nc.dram_tensor("preds", [SPC, C, N], mybir.dt.float32, kind="ExternalInput")
    targs = nc.dram_tensor("targs", [SPC, N, C], mybir.dt.float32, kind="ExternalInput")
    out = nc.dram_tensor(
        "gr_out", [QP, NG * X * GS], mybir.dt.float32, kind="ExternalOutput"
    )

    coeff_v = coeff[:].rearrange("s (p f) -> p s f", p=P)
    preds_v = preds[:].rearrange("s j (p f) -> p s j f", p=P)
    targs_v = targs[:].rearrange("s (p f) m -> p s f m", p=P)

    with tile.TileContext(nc) as tc, ExitStack() as ctx:
        us = ctx.enter_context(tc.tile_pool(name="us", bufs=2))
        t16s = ctx.enter_context(tc.tile_pool(name="t16s", bufs=2))
        a16s = ctx.enter_context(tc.tile_pool(name="a16s", bufs=2))
        w16s = ctx.enter_context(tc.tile_pool(name="w16s", bufs=2))
        outs = ctx.enter_context(tc.tile_pool(name="outs", bufs=1))
        psums = ctx.enter_context(tc.tile_pool(name="psums", bufs=2, space="PSUM"))

        out_stage = outs.tile([QP, NG * X * GS], mybir.dt.float32)

        for g in range(NG):
            sl = slice(g * GS, (g + 1) * GS)

            # all inputs cast fp32->bf16 during DMA (SWDGE); src runs are
            # 512B (coeff, preds) / 2KB (targs) contiguous
            a16 = a16s.tile([P, GS, F], mybir.dt.bfloat16, tag="a16")
            nc.gpsimd.dma_start(out=a16[:], in_=coeff_v[:, sl, :])

            t16 = t16s.tile([P, GS, F, C], mybir.dt.bfloat16, tag="t16")
            nc.gpsimd.dma_start(out=t16[:], in_=targs_v[:, sl, :, :])

            # U is x-major so each per-class preds DMA lands contiguous
            # (the DMA AP balancer only allows partition + 2 merged dims)
            u16 = us.tile([P, X, GS, F], mybir.dt.bfloat16, tag="u16")
            for j in range(C):
                nc.gpsimd.dma_start(
                    out=u16[:, j, :, :], in_=preds_v[:, sl, j, :]
                )

            # transpose targets (f,m) -> (m,f) into U; alternate engine to
            # split the strided-copy cost between DVE and GpSimd
            tr_engine = nc.vector if g % 2 == 0 else nc.gpsimd
            tr_engine.tensor_copy(
                u16[:, C:X, :, :],
                t16[:].rearrange("p s f m -> p m s f"),
            )

            # W = a * p in bf16, layout [P, i, s, f]: f-contiguous DVE writes
            w16 = w16s.tile([P, C, GS, F], mybir.dt.bfloat16, tag="w16")
            for i in range(C):
                nc.vector.tensor_mul(
                    w16[:, i, :, :],
                    a16[:],
                    u16[:, i, :, :],
                )

            psum_u = psums.tile([QP, GS * X], mybir.dt.float32, tag="pu")

            for f in range(F):
                nc.tensor.matmul(
                    psum_u[:],
                    w16[:, :, :, f],   # [128, (i, s)] stationary
                    u16[:, :, :, f].rearrange("p x s -> p s x"),  # [128,(s,x)]
                    start=(f == 0),
                    stop=(f == F - 1),
                )

            nc.scalar.copy(
                out=out_stage[:, g * (X * GS) : (g + 1) * (X * GS)], in_=psum_u[:]
            )

        nc.sync.dma_start(out=out[:], in_=out_stage[:])

    if not nc.is_finalized():
        nc.finalize()
    return nc


def _get_nc():
    if "nc" not in _CACHE:
        _CACHE["nc"] = _build_bass()
    return _CACHE["nc"]


def kernel(coefficients, predictions, targets):
    co = np.ascontiguousarray(np.asarray(coefficients, dtype=np.float32))
    pr = np.ascontiguousarray(np.asarray(predictions, dtype=np.float32))
    tg = np.ascontiguousarray(np.asarray(targets, dtype=np.float32))
    assert co.shape == (B, N) and pr.shape == (B, C, N) and tg.shape == (B, N, C)

    nc = _get_nc()
    in_maps = []
    for c in range(NCORES):
        sl = slice(c * SPC, (c + 1) * SPC)
        in_maps.append({"coeff": co[sl], "preds": pr[sl], "targs": tg[sl]})

    res = run_bass_kernel_spmd(nc, in_maps, core_ids=list(range(NCORES)))
    _CACHE["last"] = res

    # host epilogue: extract per-sample 4x4 G/R block diagonals, fp64 solve
    G = np.empty((B, C, C), np.float64)
    R = np.empty((B, C, C), np.float64)
    for c in range(NCORES):
        o = np.asarray(res.results[c]["gr_out"], dtype=np.float64)
        for g in range(NG):
            blk = o[:, g * (X * GS) : (g + 1) * (X * GS)].reshape(C, GS, GS, X)
            s0 = c * SPC + g * GS
            G[s0 : s0 + GS] = np.einsum("issj->sij", blk[:, :, :, 0:C])
            R[s0 : s0 + GS] = np.einsum("issm->sim", blk[:, :, :, C:X])

    G = 0.5 * (G + np.swapaxes(G, 1, 2))
    Xs = np.linalg.solve(G, R)
    val = (H * H) * np.einsum("bim,bim->b", R, Xs)
    loss = np.mean((4.0 - val) / 4.0)
    return np.float32(loss)


# revision 10
# speedup vs baseline: 1.1834x; 1.1834x over previous
"""Trainium2 Bass kernel for nn_CustomLoss_69999376990919.

Math: the reference's A-inner-product modified Gram-Schmidt + projection
collapses to per-sample 4x4 Gram matrices
    G[s] = P_s diag(a_s) P_s^T,   R[s] = P_s diag(a_s) T_s
after which   loss = mean_s (4 - tr(R^T G^{-1} R)) / 4
(Cholesky of G == Gram-Schmidt in exact arithmetic; <v,Av> > 0 always holds
since coefficients > 0).  The device streams all inputs (memory-bound) and
produces G/R; the tiny 4x4 solves run on the host in float64.

Sharding: pure data parallelism, batch axis 0 split across 8 cores
(64 samples each).  Per core, samples run in 4 groups of 16 so compute can
chase the (FIFO, in-order) SWDGE DMA stream group by group.
Layout: n = p*128 + f (p = SBUF partition, f = free chunk).  Per f-chunk,
a bf16 matmul pair accumulates G and R for all 16 samples jointly:
  lhsT = W(f) = (a*P)(f) as [128, (i,s)] stationary,
  rhs  = P(f) / T(f) as [128, (s,j)] moving,
  PSUM[(i,s), (s',j)] accumulated over the 128 f-chunks; the s==s' block
diagonals are the per-sample G/R entries (extracted on host).
Per group the DMAs are ordered a, p, t and the matmuls run as a G-phase
then an R-phase, so the G matmuls overlap the t-DMA and only the final
group's R-phase is exposed after the last DMA.  bf16 is safe: the loss is
1 - O(1e-4); bf16-quantized inputs move the final scalar by ~1e-9 relative.
"""

import os
from contextlib import ExitStack

import numpy as np

import concourse.bacc as bacc
import concourse.bass as bass
import concourse.tile as tile
from concourse import mybir
from concourse.bass_utils import run_bass_kernel_spmd

B, C, N = 512, 4, 16384
H = 0.0078125  # grid spacing; A = diag(h^2 * coefficients)
NCORES = 8
SPC = B // NCORES  # 64 samples per core
GS = 16            # samples per group
NG = SPC // GS     # 4 groups per core
P = 128            # SBUF partitions; n = p*128 + f
F = N // P         # 128 f-chunks
QP = C * GS        # psum partitions (i, s)

_CACHE = {}


def _build_bass():
    nc = bacc.Bacc(trn_type="TRN2")
    coeff = nc.dram_tensor("coeff", [SPC, N], mybir.dt.float32, kind="ExternalInput")
    preds = nc.dram_tensor("preds", [SPC, C, N], mybir.dt.float32, kind="ExternalInput")
    targs = nc.dram_tensor("targs", [SPC, N, C], mybir.dt.float32, kind="ExternalInput")
    out = nc.dram_tensor(
        "gr_out", [QP, NG * 2 * C * GS], mybir.dt.float32, kind="ExternalOutput"
    )

    coeff_v = coeff[:].rearrange("s (p f) -> p s f", p=P)
    preds_v = preds[:].rearrange("s j (p f) -> p s j f", p=P)
    targs_v = targs[:].rearrange("s (p f) m -> p s f m", p=P)

    with tile.TileContext(nc) as tc, ExitStack() as ctx:
        p16s = ctx.enter_context(tc.tile_pool(name="p16s", bufs=2))
        t16s = ctx.enter_context(tc.tile_pool(name="t16s", bufs=2))
        a16s = ctx.enter_context(tc.tile_pool(name="a16s", bufs=2))
        w16s = ctx.enter_context(tc.tile_pool(name="w16s", bufs=2))
        outs = ctx.enter_context(tc.tile_pool(name="outs", bufs=1))
        psums = ctx.enter_context(tc.tile_pool(name="psums", bufs=2, space="PSUM"))

        out_stage = outs.tile([QP, NG * 2 * C * GS], mybir.dt.float32)

        for g in range(NG):
            sl = slice(g * GS, (g + 1) * GS)

            # inputs cast fp32->bf16 during DMA (SWDGE, FIFO per queue);
            # order a, p, t so W and the G-phase can start before t lands
            a16 = a16s.tile([P, GS, F], mybir.dt.bfloat16, tag="a16")
            nc.gpsimd.dma_start(out=a16[:], in_=coeff_v[:, sl, :])

            p16 = p16s.tile([P, GS, C, F], mybir.dt.bfloat16, tag="p16")
            nc.gpsimd.dma_start(out=p16[:], in_=preds_v[:, sl, :, :])

            t16 = t16s.tile([P, GS, F, C], mybir.dt.bfloat16, tag="t16")
            nc.gpsimd.dma_start(out=t16[:], in_=targs_v[:, sl, :, :])

            # W = a * p in bf16, layout [P, i, s, f]: f-contiguous DVE writes
            w16 = w16s.tile([P, C, GS, F], mybir.dt.bfloat16, tag="w16")
            for i in range(C):
                nc.vector.tensor_mul(
                    w16[:, i, :, :],
                    a16[:],
                    p16[:, :, i, :],
                )

            psum_g = psums.tile([QP, GS * C], mybir.dt.float32, tag="pg")
            psum_r = psums.tile([QP, GS * C], mybir.dt.float32, tag="pr")

            for f in range(F):
                nc.tensor.matmul(
                    psum_g[:],
                    w16[:, :, :, f],   # [128, (i, s)] stationary
                    p16[:, :, :, f],   # [128, (s, j)] moving
                    start=(f == 0),
                    stop=(f == F - 1),
                )
            for f in range(F):
                nc.tensor.matmul(
                    psum_r[:],
                    w16[:, :, :, f],
                    t16[:, :, f, :],   # [128, (s, m)] moving
                    start=(f == 0),
                    stop=(f == F - 1),
                )

            gw = 2 * C * GS  # out_stage columns per group
            nc.scalar.copy(
                out=out_stage[:, g * gw : g * gw + C * GS], in_=psum_g[:]
            )
            nc.scalar.copy(
                out=out_stage[:, g * gw + C * GS : (g + 1) * gw], in_=psum_r[:]
            )

        nc.sync.dma_start(out=out[:], in_=out_stage[:])

    if not nc.is_finalized():
        nc.finalize()
    return nc


def _get_nc():
    if "nc" not in _CACHE:
        _CACHE["nc"] = _build_bass()
    return _CACHE["nc"]


def kernel(coefficients, predictions, targets):
    co = np.ascontiguousarray(np.asarray(coefficients, dtype=np.float32))
    pr = np.ascontiguousarray(np.asarray(predictions, dtype=np.float32))
    tg = np.ascontiguousarray(np.asarray(targets, dtype=np.float32))
    assert co.shape == (B, N) and pr.shape == (B, C, N) and tg.shape == (B, N, C)

    nc = _get_nc()
    in_maps = []
    for c in range(NCORES):
        sl = slice(c * SPC, (c + 1) * SPC)
        in_maps.append({"coeff": co[sl], "preds": pr[sl], "targs": tg[sl]})

    res = run_bass_kernel_spmd(nc, in_maps, core_ids=list(range(NCORES)))
    _CACHE["last"] = res

    # host epilogue: extract per-sample 4x4 G/R block diagonals, fp64 solve
    G = np.empty((B, C, C), np.float64)
    R = np.empty((B, C, C), np.float64)
    gw = 2 * C * GS
    for c in range(NCORES):
        o = np.asarray(res.results[c]["gr_out"], dtype=np.float64)
        for g in range(NG):
            bg = o[:, g * gw : g * gw + C * GS].reshape(C, GS, GS, C)
            br = o[:, g * gw + C * GS : (g + 1) * gw].reshape(C, GS, GS, C)
            s0 = c * SPC + g * GS
            G[s0 : s0 + GS] = np.einsum("issj->sij", bg)
            R[s0 : s0 + GS] = np.einsum("issm->sim", br)

    G = 0.5 * (G + np.swapaxes(G, 1, 2))
    Xs = np.linalg.solve(G, R)
    val = (H * H) * np.einsum("bim,bim->b", R, Xs)
    loss = np.mean((4.0 - val) / 4.0)
    return np.float32(loss)
